# revision 10
# baseline (speedup 1.0000x reference)
"""GCN encoder (2x GCNConv + linear projection, relu) on 8 Trainium2 cores.

Self-contained: hardcodes the problem shapes (N=50000, E=800000, C=128,
OUT_C=64) and the sharding strategy.  Host side does structural prep only
(edge partitioning/sorting/padding, index-list construction); all FP math
(matmuls, rsqrt, scaling, aggregation, bias, relu) runs on device.

Math identity used on device, per GCNConv layer:
    g = dinv * (x @ W.T)          (dinv = rsqrt(indeg+1), per node)
    out[d] = relu(dinv[d] * (sum_{e: dst=d} g[src_e] + g[d]) + b)

Device mapping per core (v3):
  - nodes sharded by contiguous range (6250/core, padded to 6272 = 49
    windows of 128)
  - per layer the g table is built COLLECTIVELY: each core computes only
    its own 49 windows of g rows (local matmul + dinv scale); two
    AllGather collectives assemble two "piece" tables in DRAM:
      piece0 = all cores' windows 0..24  (8*3200 = 25600 rows)
      piece1 = all cores' windows 25..48 (8*3072 = 24576 rows)
    Both < 32768 rows so int16 gather indices cover them, and piece0 is
    ready early so its gathers overlap the piece1 collective.
  - edges partitioned by dst owner; per layer processed in two phases
    (piece0 edges, then piece1 edges).  Within a phase, windows are
    processed in groups of 4; ONE dma_gather per (group, piece) emitted
    as PREPARE_ONLY + trigger_dma so the GpSimd engine is never held
    through the transfer (descriptor generation runs even before the
    table collective lands; the trigger carries that dep), and calls
    rotate over the 4 SWDGE queues so their drains proceed concurrently.
  - segment-sum per window: selection-matrix (tensor_scalar is_equal vs
    iota) matmuls accumulating into a [128 dst x 128 feat] fp32 PSUM
    tile; phase-0 result (+ self term g[d]) is flushed to SBUF, phase-1
    re-accumulates and applies the epilogue.
  - PE tail work (transpose + next-layer table row / projection) is
    batched at group end so it does not stall the in-order PE queue
    behind each window's DVE epilogue.
"""

import sys
import numpy as np

for _p in ("/opt/trn_rl_repo",):
    if _p not in sys.path:
        sys.path.append(_p)

import concourse.bacc as bacc
import concourse.tile as tile
from concourse import bass, mybir, bass_utils

F32 = mybir.dt.float32
BF16 = mybir.dt.bfloat16
I16 = mybir.dt.int16
AF = mybir.ActivationFunctionType
ALU = mybir.AluOpType
NP_BF16 = mybir.dt.np(BF16)

N = 50000
E = 800000
C = 128
OUT_C = 64
CORES = 8
S = N // CORES            # 6250 real nodes per shard
NW = 49                   # windows of 128 dst nodes per core
SP = NW * 128             # 6272 padded shard rows
W0 = 25                   # windows in piece 0
W1 = NW - W0              # windows in piece 1
PR0 = W0 * 128            # 3200 piece-0 rows per core
PR1 = W1 * 128            # 3072
T0 = CORES * PR0          # 25600 piece-0 table rows
T1 = CORES * PR1          # 24576
WG = 4                    # windows per gather group
GROUPS = [list(range(s, min(s + WG, NW))) for s in range(0, NW, WG)]
NG = len(GROUPS)          # 13


def _wrap16(a):
    """[L] -> [128, L/16] int16 idx layout for dma_gather (16-wrap, 8x repl)."""
    assert a.size % 16 == 0
    w = a.reshape(-1, 16).T.astype(np.int16)
    return np.ascontiguousarray(np.tile(w, (8, 1)))


def _host_prep(x, edge_index):
    """Structural prep: edge partitioning/sorting/padding + index lists."""
    src = np.asarray(edge_index[0]).astype(np.int64)
    dst = np.asarray(edge_index[1]).astype(np.int64)
    deg = np.bincount(dst, minlength=N).astype(np.float32) + 1.0

    owner = dst // S
    loc = dst - owner * S
    win = loc // 128
    rel = (loc % 128).astype(np.float32)
    sc = src // S
    sl = src - sc * S
    pc = (sl >= PR0).astype(np.int64)            # source piece
    prow = np.where(pc == 1, sc * PR1 + sl - PR0, sc * PR0 + sl)

    # common chunk schedule: caps[p][w] = chunks per (window, piece),
    # max over cores so the single SPMD program fits every core
    key = ((owner * NW + win) * 2 + pc)
    counts = np.bincount(key, minlength=CORES * NW * 2).reshape(CORES, NW, 2)
    maxc = counts.max(axis=0)                                  # [NW, 2]
    caps = [[-(-int(maxc[w, p]) // 128) for w in range(NW)] for p in (0, 1)]
    assert all(cc >= 1 for p in (0, 1) for cc in caps[p])

    # per-(group, piece) gather call sizes (blocks) and idx column offsets
    gnb = [[sum(caps[p][w] for w in ws) for ws in GROUPS] for p in (0, 1)]
    off16 = {}
    o = 0
    for p in (0, 1):
        for g in range(NG):
            off16[(g, p)] = o
            o += gnb[p][g] * 8            # blocks*128/16 idx cols
    tot16 = o

    degp = np.ones(SP * CORES, np.float32)
    nodes = np.arange(N, dtype=np.int64)
    degp[(nodes // S) * SP + (nodes % S)] = deg

    per_core = []
    for c in range(CORES):
        m = owner == c
        cw, cr, cs, cp = win[m], rel[m], prow[m], pc[m]
        order = np.lexsort((cs, cp, cw))
        cw, cr, cs, cp = cw[order], cr[order], cs[order], cp[order]
        k = cw * 2 + cp
        idx_parts = [[], []]
        rel_parts = [[], []]
        for w in range(NW):
            for p in (0, 1):
                lo = np.searchsorted(k, w * 2 + p, "left")
                hi = np.searchsorted(k, w * 2 + p, "right")
                n = hi - lo
                cap = caps[p][w]
                assert n <= cap * 128
                iv = np.concatenate(
                    [cs[lo:hi], np.zeros(cap * 128 - n, np.int64)])
                rv = np.concatenate(
                    [cr[lo:hi], np.full(cap * 128 - n, -1.0, np.float32)])
                idx_parts[p].append(iv)
                rel_parts[p].append(rv)
        # idx stream in call order: (p, group)
        idx_all = []
        for p in (0, 1):
            for ws in GROUPS:
                for w in ws:
                    idx_all.append(idx_parts[p][w])
        idx_all = np.concatenate(idx_all)
        # rel stream in chunk-consumption order: phase 0 windows, phase 1
        rel_all = np.concatenate(rel_parts[0] + rel_parts[1])
        relT = np.ascontiguousarray(rel_all.reshape(-1, 128).T)
        degl = np.ascontiguousarray(
            degp[c * SP:(c + 1) * SP].reshape(NW, 128).T)        # [128, NW]
        xpad = np.zeros((SP, C), np.float32)
        xpad[:S] = np.asarray(x, np.float32)[c * S:(c + 1) * S]
        xtl = np.ascontiguousarray(xpad.T).astype(NP_BF16)       # [128, SP]
        per_core.append(dict(idx=_wrap16(idx_all), rel=relT, degl=degl,
                             xtl=xtl))

    sched = dict(caps=caps, gnb=gnb, off16=off16, tot16=tot16)
    return sched, per_core


def _build_nc(sched):
    caps, gnb, off16, tot16 = (sched["caps"], sched["gnb"],
                               sched["off16"], sched["tot16"])
    nb_max = [max(gnb[0]), max(gnb[1])]
    nchcol = sum(caps[0]) + sum(caps[1])

    nc = bacc.Bacc("TRN2", target_bir_lowering=False, debug=False,
                   enable_asserts=False, num_devices=CORES,
                   num_swdge_queues=4)

    def inp(name, shape, dt=F32):
        return nc.dram_tensor(name, shape, dt, kind="ExternalInput").ap()

    xtl_d = inp("xtl", [128, SP], BF16)
    w1t_d = inp("w1t", [C, C], BF16)
    w2t_d = inp("w2t", [C, C], BF16)
    wpt_d = inp("wpt", [C, OUT_C], BF16)
    b1b_d = inp("b1b", [128, C])
    b2b_d = inp("b2b", [128, C])
    bpb_d = inp("bpb", [128, OUT_C])
    degl_d = inp("degl", [128, NW])
    iota_d = inp("iota", [128, 128], BF16)
    ident_d = inp("ident", [128, 128])
    idx_d = inp("idx", [128, tot16], I16)
    rel_d = inp("rel", [128, nchcol])
    out_d = nc.dram_tensor("out", [SP, OUT_C], F32, kind="ExternalOutput").ap()

    gloc = {}
    gtab = {}
    for L in (1, 2):
        for p, (pr, tr) in enumerate(((PR0, T0), (PR1, T1))):
            gloc[(L, p)] = nc.dram_tensor(
                f"g{L}loc{p}", [pr, C], BF16, kind="Internal").ap()
            gtab[(L, p)] = nc.dram_tensor(
                f"g{L}tab{p}", [tr, C], BF16, kind="Internal",
                addr_space="Shared").ap()

    from contextlib import ExitStack
    with tile.TileContext(nc) as tc, ExitStack() as ctx:
        cp = ctx.enter_context(tc.tile_pool(name="consts", bufs=1))
        stg0 = ctx.enter_context(tc.tile_pool(name="stg0", bufs=4))
        stg1 = ctx.enter_context(tc.tile_pool(name="stg1", bufs=4))
        spool = ctx.enter_context(tc.tile_pool(name="sel", bufs=8))
        epool = ctx.enter_context(tc.tile_pool(name="epi", bufs=4))
        opool = ctx.enter_context(tc.tile_pool(name="otiles", bufs=1))
        opool2 = ctx.enter_context(tc.tile_pool(name="owin", bufs=6))
        ppw = ctx.enter_context(tc.tile_pool(name="psw", bufs=2, space="PSUM"))
        ppg = ctx.enter_context(tc.tile_pool(name="psg", bufs=2, space="PSUM"))
        ppt = ctx.enter_context(tc.tile_pool(name="pst", bufs=2, space="PSUM"))
        ppp = ctx.enter_context(tc.tile_pool(name="psp", bufs=1, space="PSUM"))

        def cload(name, ap, shape, dt=F32):
            t = cp.tile(shape, dt, tag=name)
            nc.sync.dma_start(t[:], ap[:])
            return t

        # build-critical consts first so the first collective fires ASAP
        degl = cload("degl", degl_d, [128, NW])
        xtl = cload("xtl", xtl_d, [128, SP], BF16)
        w1t = cload("w1t", w1t_d, [C, C], BF16)
        w2t = cload("w2t", w2t_d, [C, C], BF16)
        sql = cp.tile([128, NW], F32, tag="sql")
        nc.scalar.activation(sql[:], degl[:], AF.Sqrt)
        dinvl = cp.tile([128, NW], F32, tag="dinvl")
        nc.vector.reciprocal(dinvl[:], sql[:])

        # ---- L1 local table rows: g1 = dinv * (X_local @ W1.T) ----
        gl = {}
        for w in range(NW):
            ps = ppg.tile([128, C], F32, tag="psg")
            nc.tensor.matmul(ps[:], lhsT=xtl[:, w * 128:(w + 1) * 128],
                             rhs=w1t[:], start=True, stop=True)
            gb = epool.tile([128, C], BF16, tag="gb")
            nc.vector.tensor_tensor(
                out=gb[:], in0=ps[:],
                in1=dinvl[:, w:w + 1].to_broadcast([128, 128]), op=ALU.mult)
            p = int(w >= W0)
            r0 = (w - (W0 if p else 0)) * 128
            nc.sync.dma_start(gloc[(1, p)][r0:r0 + 128, :], gb[:])
            glw = opool.tile([128, C], F32, tag=f"gl_{w}")
            nc.scalar.activation(glw[:], ps[:], AF.Identity,
                                 scale=dinvl[:, w:w + 1])
            gl[w] = glw
            if w == W0 - 1:
                nc.gpsimd.collective_compute(
                    "AllGather", ALU.bypass,
                    replica_groups=[list(range(CORES))],
                    ins=[gloc[(1, 0)][:]], outs=[gtab[(1, 0)][:]])
        nc.gpsimd.collective_compute(
            "AllGather", ALU.bypass, replica_groups=[list(range(CORES))],
            ins=[gloc[(1, 1)][:]], outs=[gtab[(1, 1)][:]])

        # remaining consts (overlap the first collective)
        wpt = cload("wpt", wpt_d, [C, OUT_C], BF16)
        b1b = cload("b1b", b1b_d, [128, C])
        b2b = cload("b2b", b2b_d, [128, C])
        bpb = cload("bpb", bpb_d, [128, OUT_C])
        iota = cload("iota", iota_d, [128, 128], BF16)
        ident = cload("ident", ident_d, [128, 128])
        idx = cload("idx", idx_d, [128, tot16], I16)
        rel = cload("rel", rel_d, [128, nchcol])

        state = dict(ci=0, qi=0)
        parts = {}

        def sel_for(ci):
            sel = spool.tile([128, 128], BF16, tag="sel")
            nc.vector.tensor_scalar(out=sel[:], in0=iota[:],
                                    scalar1=rel[:, ci:ci + 1], scalar2=None,
                                    op0=ALU.is_equal)
            return sel

        def phase(L, p, post_window, tail=None):
            """One gather+aggregate phase: piece p edges of layer L."""
            if p == 0:
                state["ci"] = 0          # rel stream is shared by both layers
            pool = stg0 if p == 0 else stg1
            for g, ws in enumerate(GROUPS):
                nb = gnb[p][g]
                stg = pool.tile([128, nb_max[p], C], BF16, tag="stg")
                nidx = nb * 128
                o16 = off16[(g, p)]
                q = state["qi"] % 4
                state["qi"] += 1
                nc.gpsimd.dma_gather(
                    stg[:, :nb, :], gtab[(L, p)],
                    idx[:, o16:o16 + nidx // 16], nidx, nidx, elem_size=C,
                    single_packet=False, queue_num=q)
                blk = 0
                os = []
                for w in ws:
                    cap = caps[p][w]
                    ps = ppw.tile([128, C], F32, tag="psw")
                    for j in range(cap):
                        sel = sel_for(state["ci"])
                        state["ci"] += 1
                        nc.tensor.matmul(ps[:], lhsT=sel[:], rhs=stg[:, blk, :],
                                         start=(j == 0), stop=(j == cap - 1))
                        blk += 1
                    os.append((w, post_window(w, ps)))
                if tail is not None:
                    for w, o in os:
                        tail(w, o)

        def flush_partial(w, ps):
            """Phase-0 epilogue: partial = psum + self-term g[d]."""
            part = opool.tile([128, C], F32, tag=f"part_{w}")
            nc.vector.tensor_tensor(out=part[:], in0=ps[:], in1=gl[w][:],
                                    op=ALU.add)
            parts[w] = part

        def final_epilogue(w, ps, bias_sb):
            """Phase-1 epilogue: relu(dinv*(ps + partial) + bias)."""
            t1 = epool.tile([128, C], F32, tag="t1")
            nc.vector.tensor_tensor(out=t1[:], in0=ps[:], in1=parts[w][:],
                                    op=ALU.add)
            t2 = epool.tile([128, C], F32, tag="t2")
            nc.vector.tensor_tensor(
                out=t2[:], in0=t1[:],
                in1=dinvl[:, w:w + 1].to_broadcast([128, 128]), op=ALU.mult)
            t3 = epool.tile([128, C], F32, tag="t3")
            nc.vector.tensor_tensor(out=t3[:], in0=t2[:], in1=bias_sb[:],
                                    op=ALU.add)
            o = opool2.tile([128, C], F32, tag="otile")
            nc.scalar.activation(o[:], t3[:], AF.Relu)
            return o

        def l1_tail(w, o):
            """Transpose h1 and build the L2 table row for window w."""
            pst = ppt.tile([128, 128], F32, tag="pst")
            nc.tensor.transpose(pst[:], o[:], ident[:])
            h1t = epool.tile([128, 128], BF16, tag="h1t")
            nc.vector.tensor_copy(h1t[:], pst[:])
            ps2 = ppg.tile([128, C], F32, tag="psg")
            nc.tensor.matmul(ps2[:], lhsT=h1t[:], rhs=w2t[:],
                             start=True, stop=True)
            gb = epool.tile([128, C], BF16, tag="gb")
            nc.vector.tensor_tensor(
                out=gb[:], in0=ps2[:],
                in1=dinvl[:, w:w + 1].to_broadcast([128, 128]), op=ALU.mult)
            p = int(w >= W0)
            r0 = (w - (W0 if p else 0)) * 128
            nc.sync.dma_start(gloc[(2, p)][r0:r0 + 128, :], gb[:])
            glw = opool.tile([128, C], F32, tag=f"gl_{w}")
            nc.scalar.activation(glw[:], ps2[:], AF.Identity,
                                 scale=dinvl[:, w:w + 1])
            gl[w] = glw
            if w == W0 - 1:
                nc.gpsimd.collective_compute(
                    "AllGather", ALU.bypass,
                    replica_groups=[list(range(CORES))],
                    ins=[gloc[(2, 0)][:]], outs=[gtab[(2, 0)][:]])
            if w == NW - 1:
                nc.gpsimd.collective_compute(
                    "AllGather", ALU.bypass,
                    replica_groups=[list(range(CORES))],
                    ins=[gloc[(2, 1)][:]], outs=[gtab[(2, 1)][:]])

        def l2_tail(w, o):
            """Project and store output rows for window w."""
            pst = ppt.tile([128, 128], F32, tag="pst")
            nc.tensor.transpose(pst[:], o[:], ident[:])
            h2t = epool.tile([128, 128], BF16, tag="h2t")
            nc.vector.tensor_copy(h2t[:], pst[:])
            psp = ppp.tile([128, OUT_C], F32, tag="psp")
            nc.tensor.matmul(psp[:], lhsT=h2t[:], rhs=wpt[:],
                             start=True, stop=True)
            of = epool.tile([128, OUT_C], F32, tag="of")
            nc.vector.tensor_tensor(out=of[:], in0=psp[:], in1=bpb[:],
                                    op=ALU.add)
            ofr = epool.tile([128, OUT_C], F32, tag="ofr")
            nc.scalar.activation(ofr[:], of[:], AF.Relu)
            nc.sync.dma_start(out_d[w * 128:(w + 1) * 128, :], ofr[:])

        phase(1, 0, flush_partial)
        phase(1, 1, lambda w, ps: final_epilogue(w, ps, b1b), l1_tail)
        phase(2, 0, flush_partial)
        phase(2, 1, lambda w, ps: final_epilogue(w, ps, b2b), l2_tail)

    nc.compile()
    return nc


def _make_in_maps(sched, per_core, W1, b1, W2, b2, Wp, bp):
    w1t = np.ascontiguousarray(np.asarray(W1, np.float32).T).astype(NP_BF16)
    w2t = np.ascontiguousarray(np.asarray(W2, np.float32).T).astype(NP_BF16)
    wpt = np.ascontiguousarray(np.asarray(Wp, np.float32).T).astype(NP_BF16)
    b1b = np.ascontiguousarray(np.tile(np.asarray(b1, np.float32)[None], (128, 1)))
    b2b = np.ascontiguousarray(np.tile(np.asarray(b2, np.float32)[None], (128, 1)))
    bpb = np.ascontiguousarray(np.tile(np.asarray(bp, np.float32)[None], (128, 1)))
    iota = np.ascontiguousarray(
        np.tile(np.arange(128, dtype=np.float32)[None, :],
                (128, 1))).astype(NP_BF16)
    ident = np.eye(128, dtype=np.float32)
    base = dict(w1t=w1t, w2t=w2t, wpt=wpt, b1b=b1b, b2b=b2b, bpb=bpb,
                iota=iota, ident=ident)
    in_maps = []
    for c in range(CORES):
        pc = per_core[c]
        m = dict(base)
        m["idx"] = pc["idx"]
        m["rel"] = pc["rel"]
        m["degl"] = pc["degl"]
        m["xtl"] = pc["xtl"]
        in_maps.append(m)
    return in_maps


def _run(inputs, trace=False, tmpdir=None, verbose=True):
    import time
    t0 = time.time()
    def _log(msg):
        if verbose:
            print(f"[kernel {time.time()-t0:7.1f}s] {msg}", flush=True)
    sched, per_core = _host_prep(inputs["x"], inputs["edge_index"])
    _log("host prep done")
    nc = _build_nc(sched)
    _log("build+compile done")
    in_maps = _make_in_maps(sched, per_core,
                            inputs["W1"], inputs["b1"], inputs["W2"],
                            inputs["b2"], inputs["Wp"], inputs["bp"])
    _log("in_maps done")
    core_ids = list(range(CORES))
    if trace:
        # NTFF profiling needs a warm first execute; run once untraced.
        bass_utils.run_bass_kernel_spmd(nc, in_maps, core_ids=core_ids,
                                        trace=False)
        _log("warmup run done")
    res = bass_utils.run_bass_kernel_spmd(nc, in_maps, core_ids=core_ids,
                                          trace=trace, tmpdir=tmpdir)
    _log("run done")
    out = np.empty((N, OUT_C), np.float32)
    for c in range(CORES):
        out[c * S:(c + 1) * S] = res.results[c]["out"][:S]
    return out, res


def kernel(**inputs):
    out, _ = _run(inputs)
    return out


# revision 12
# speedup vs baseline: 1.4932x; 1.4932x over previous
"""GCN encoder (2x GCNConv + linear projection, relu) on 8 Trainium2 cores.

Self-contained: hardcodes the problem shapes (N=50000, E=800000, C=128,
OUT_C=64) and the sharding strategy.  Host side does structural prep only
(edge partitioning/sorting/padding, index-list construction); all FP math
(matmuls, rsqrt, scaling, aggregation, bias, relu) runs on device.

Math identity used on device, per GCNConv layer:
    g = dinv * (x @ W.T)          (dinv = rsqrt(indeg+1), per node)
    out[d] = relu(dinv[d] * (sum_{e: dst=d} g[src_e] + g[d]) + b)

Device mapping per core (v3):
  - nodes sharded by contiguous range (6250/core, padded to 6272 = 49
    windows of 128)
  - per layer the g table is built COLLECTIVELY: each core computes only
    its own 49 windows of g rows (local matmul + dinv scale); two
    AllGather collectives assemble two "piece" tables in DRAM:
      piece0 = all cores' windows 0..24  (8*3200 = 25600 rows)
      piece1 = all cores' windows 25..48 (8*3072 = 24576 rows)
    Both < 32768 rows so int16 gather indices cover them, and piece0 is
    ready early so its gathers overlap the piece1 collective.
  - edges partitioned by dst owner; per layer processed in two phases
    (piece0 edges, then piece1 edges).  Within a phase, windows are
    processed in groups of 4; ONE dma_gather per (group, piece) emitted
    as PREPARE_ONLY + trigger_dma so the GpSimd engine is never held
    through the transfer (descriptor generation runs even before the
    table collective lands; the trigger carries that dep), and calls
    rotate over the 4 SWDGE queues so their drains proceed concurrently.
  - segment-sum per window: selection-matrix (tensor_scalar is_equal vs
    iota) matmuls accumulating into a [128 dst x 128 feat] fp32 PSUM
    tile; phase-0 result (+ self term g[d]) is flushed to SBUF, phase-1
    re-accumulates and applies the epilogue.
  - PE tail work (transpose + next-layer table row / projection) is
    batched at group end so it does not stall the in-order PE queue
    behind each window's DVE epilogue.
"""

import sys
import numpy as np

for _p in ("/opt/trn_rl_repo",):
    if _p not in sys.path:
        sys.path.append(_p)

import concourse.bacc as bacc
import concourse.tile as tile
from concourse import bass, mybir, bass_utils

F32 = mybir.dt.float32
BF16 = mybir.dt.bfloat16
I16 = mybir.dt.int16
AF = mybir.ActivationFunctionType
ALU = mybir.AluOpType
NP_BF16 = mybir.dt.np(BF16)

N = 50000
E = 800000
C = 128
OUT_C = 64
CORES = 8
S = N // CORES            # 6250 real nodes per shard
NW = 49                   # windows of 128 dst nodes per core
SP = NW * 128             # 6272 padded shard rows
W0 = 25                   # windows in piece 0
W1 = NW - W0              # windows in piece 1
PR0 = W0 * 128            # 3200 piece-0 rows per core
PR1 = W1 * 128            # 3072
T0 = CORES * PR0          # 25600 piece-0 table rows
T1 = CORES * PR1          # 24576
WG = 4                    # windows per gather group
GROUPS = [list(range(s, min(s + WG, NW))) for s in range(0, NW, WG)]
NG = len(GROUPS)          # 13


def _wrap16(a):
    """[L] -> [128, L/16] int16 idx layout for dma_gather (16-wrap, 8x repl)."""
    assert a.size % 16 == 0
    w = a.reshape(-1, 16).T.astype(np.int16)
    return np.ascontiguousarray(np.tile(w, (8, 1)))


def _host_prep(x, edge_index):
    """Structural prep: edge partitioning/sorting/padding + index lists."""
    src = np.asarray(edge_index[0]).astype(np.int64)
    dst = np.asarray(edge_index[1]).astype(np.int64)
    deg = np.bincount(dst, minlength=N).astype(np.float32) + 1.0

    owner = dst // S
    loc = dst - owner * S
    win = loc // 128
    rel = (loc % 128).astype(np.float32)
    sc = src // S
    sl = src - sc * S
    pc = (sl >= PR0).astype(np.int64)            # source piece
    prow = np.where(pc == 1, sc * PR1 + sl - PR0, sc * PR0 + sl)

    # common chunk schedule: caps[p][w] = chunks per (window, piece),
    # max over cores so the single SPMD program fits every core
    key = ((owner * NW + win) * 2 + pc)
    counts = np.bincount(key, minlength=CORES * NW * 2).reshape(CORES, NW, 2)
    maxc = counts.max(axis=0)                                  # [NW, 2]
    caps = [[-(-int(maxc[w, p]) // 128) for w in range(NW)] for p in (0, 1)]
    assert all(cc >= 1 for p in (0, 1) for cc in caps[p])

    # per-(group, piece) gather call sizes (blocks) and idx column offsets
    gnb = [[sum(caps[p][w] for w in ws) for ws in GROUPS] for p in (0, 1)]
    off16 = {}
    o = 0
    for p in (0, 1):
        for g in range(NG):
            off16[(g, p)] = o
            o += gnb[p][g] * 8            # blocks*128/16 idx cols
    tot16 = o

    degp = np.ones(SP * CORES, np.float32)
    nodes = np.arange(N, dtype=np.int64)
    degp[(nodes // S) * SP + (nodes % S)] = deg

    per_core = []
    for c in range(CORES):
        m = owner == c
        cw, cr, cs, cp = win[m], rel[m], prow[m], pc[m]
        order = np.lexsort((cs, cp, cw))
        cw, cr, cs, cp = cw[order], cr[order], cs[order], cp[order]
        k = cw * 2 + cp
        idx_parts = [[], []]
        rel_parts = [[], []]
        for w in range(NW):
            for p in (0, 1):
                lo = np.searchsorted(k, w * 2 + p, "left")
                hi = np.searchsorted(k, w * 2 + p, "right")
                n = hi - lo
                cap = caps[p][w]
                assert n <= cap * 128
                iv = np.concatenate(
                    [cs[lo:hi], np.zeros(cap * 128 - n, np.int64)])
                rv = np.concatenate(
                    [cr[lo:hi], np.full(cap * 128 - n, -1.0, np.float32)])
                idx_parts[p].append(iv)
                rel_parts[p].append(rv)
        # idx stream in call order: (p, group)
        idx_all = []
        for p in (0, 1):
            for ws in GROUPS:
                for w in ws:
                    idx_all.append(idx_parts[p][w])
        idx_all = np.concatenate(idx_all)
        # rel stream in chunk-consumption order: phase 0 windows, phase 1
        rel_all = np.concatenate(rel_parts[0] + rel_parts[1])
        relT = np.ascontiguousarray(
            rel_all.reshape(-1, 128).T).astype(NP_BF16)
        degl = np.ascontiguousarray(
            degp[c * SP:(c + 1) * SP].reshape(NW, 128).T)        # [128, NW]
        xpad = np.zeros((SP, C), np.float32)
        xpad[:S] = np.asarray(x, np.float32)[c * S:(c + 1) * S]
        xtl = np.ascontiguousarray(xpad.T).astype(NP_BF16)       # [128, SP]
        per_core.append(dict(idx=_wrap16(idx_all), rel=relT, degl=degl,
                             xtl=xtl))

    sched = dict(caps=caps, gnb=gnb, off16=off16, tot16=tot16)
    return sched, per_core


def _build_nc(sched):
    caps, gnb, off16, tot16 = (sched["caps"], sched["gnb"],
                               sched["off16"], sched["tot16"])
    nb_max = [max(gnb[0]), max(gnb[1])]
    nchcol = sum(caps[0]) + sum(caps[1])

    nc = bacc.Bacc("TRN2", target_bir_lowering=False, debug=False,
                   enable_asserts=False, num_devices=CORES,
                   num_swdge_queues=4)

    def inp(name, shape, dt=F32):
        return nc.dram_tensor(name, shape, dt, kind="ExternalInput").ap()

    xtl_d = inp("xtl", [128, SP], BF16)
    w1t_d = inp("w1t", [C, C], BF16)
    w2t_d = inp("w2t", [C, C], BF16)
    wpt_d = inp("wpt", [C, OUT_C], BF16)
    b1b_d = inp("b1b", [128, C])
    b2b_d = inp("b2b", [128, C])
    bpb_d = inp("bpb", [128, OUT_C])
    degl_d = inp("degl", [128, NW])
    iota_d = inp("iota", [128, 128], BF16)
    ident_d = inp("ident", [128, 128])
    idx_d = inp("idx", [128, tot16], I16)
    rel_d = inp("rel", [128, nchcol], BF16)
    out_d = nc.dram_tensor("out", [SP, OUT_C], F32, kind="ExternalOutput").ap()

    gloc = {}
    gtab = {}
    for L in (1, 2):
        for p, (pr, tr) in enumerate(((PR0, T0), (PR1, T1))):
            gloc[(L, p)] = nc.dram_tensor(
                f"g{L}loc{p}", [pr, C], BF16, kind="Internal").ap()
            gtab[(L, p)] = nc.dram_tensor(
                f"g{L}tab{p}", [tr, C], BF16, kind="Internal",
                addr_space="Shared").ap()

    from contextlib import ExitStack
    with tile.TileContext(nc) as tc, ExitStack() as ctx:
        cp = ctx.enter_context(tc.tile_pool(name="consts", bufs=1))
        stg0 = ctx.enter_context(tc.tile_pool(name="stg0", bufs=4))
        stg1 = ctx.enter_context(tc.tile_pool(name="stg1", bufs=4))
        spool = ctx.enter_context(tc.tile_pool(name="sel", bufs=8))
        epool = ctx.enter_context(tc.tile_pool(name="epi", bufs=4))
        opool = ctx.enter_context(tc.tile_pool(name="otiles", bufs=1))
        opool2 = ctx.enter_context(tc.tile_pool(name="owin", bufs=6))
        ppw = ctx.enter_context(tc.tile_pool(name="psw", bufs=2, space="PSUM"))
        ppg = ctx.enter_context(tc.tile_pool(name="psg", bufs=2, space="PSUM"))
        ppt = ctx.enter_context(tc.tile_pool(name="pst", bufs=2, space="PSUM"))
        ppp = ctx.enter_context(tc.tile_pool(name="psp", bufs=1, space="PSUM"))

        def cload(name, ap, shape, dt=F32):
            t = cp.tile(shape, dt, tag=name)
            nc.sync.dma_start(t[:], ap[:])
            return t

        # build-critical consts first so the first collective fires ASAP
        degl = cload("degl", degl_d, [128, NW])
        xtl = cload("xtl", xtl_d, [128, SP], BF16)
        w1t = cload("w1t", w1t_d, [C, C], BF16)
        w2t = cload("w2t", w2t_d, [C, C], BF16)
        sql = cp.tile([128, NW], F32, tag="sql")
        nc.scalar.activation(sql[:], degl[:], AF.Sqrt)
        dinvl = cp.tile([128, NW], F32, tag="dinvl")
        nc.vector.reciprocal(dinvl[:], sql[:])

        # ---- L1 local table rows: g1 = dinv * (X_local @ W1.T) ----
        gl = {}
        for w in range(NW):
            ps = ppg.tile([128, C], F32, tag="psg")
            nc.tensor.matmul(ps[:], lhsT=xtl[:, w * 128:(w + 1) * 128],
                             rhs=w1t[:], start=True, stop=True)
            gb = epool.tile([128, C], BF16, tag="gb")
            nc.scalar.activation(gb[:], ps[:], AF.Identity,
                                 scale=dinvl[:, w:w + 1])
            p = int(w >= W0)
            r0 = (w - (W0 if p else 0)) * 128
            nc.sync.dma_start(gloc[(1, p)][r0:r0 + 128, :], gb[:])
            glw = opool.tile([128, C], F32, tag=f"gl_{w}")
            nc.scalar.activation(glw[:], ps[:], AF.Identity,
                                 scale=dinvl[:, w:w + 1])
            gl[w] = glw
            if w == W0 - 1:
                nc.gpsimd.collective_compute(
                    "AllGather", ALU.bypass,
                    replica_groups=[list(range(CORES))],
                    ins=[gloc[(1, 0)][:]], outs=[gtab[(1, 0)][:]])
        nc.gpsimd.collective_compute(
            "AllGather", ALU.bypass, replica_groups=[list(range(CORES))],
            ins=[gloc[(1, 1)][:]], outs=[gtab[(1, 1)][:]])

        # remaining consts (overlap the first collective)
        wpt = cload("wpt", wpt_d, [C, OUT_C], BF16)
        b1b = cload("b1b", b1b_d, [128, C])
        b2b = cload("b2b", b2b_d, [128, C])
        bpb = cload("bpb", bpb_d, [128, OUT_C])
        iota = cload("iota", iota_d, [128, 128], BF16)
        ident = cload("ident", ident_d, [128, 128])
        idx = cload("idx", idx_d, [128, tot16], I16)
        rel = cload("rel", rel_d, [128, nchcol], BF16)

        state = dict(ci=0, qi=0)
        parts = {}

        def sel_for(ci):
            sel = spool.tile([128, 128], BF16, tag="sel")
            nc.vector.tensor_tensor(
                out=sel[:], in0=rel[:, ci:ci + 1].to_broadcast([128, 128]),
                in1=iota[:], op=ALU.is_equal)
            return sel

        def phase(L, p, post_window, tail=None):
            """One gather+aggregate phase: piece p edges of layer L."""
            if p == 0:
                state["ci"] = 0          # rel stream is shared by both layers
            pool = stg0 if p == 0 else stg1
            for g, ws in enumerate(GROUPS):
                nb = gnb[p][g]
                stg = pool.tile([128, nb_max[p], C], BF16, tag="stg")
                nidx = nb * 128
                o16 = off16[(g, p)]
                q = state["qi"] % 4
                state["qi"] += 1
                nc.gpsimd.dma_gather(
                    stg[:, :nb, :], gtab[(L, p)],
                    idx[:, o16:o16 + nidx // 16], nidx, nidx, elem_size=C,
                    single_packet=False, queue_num=q)
                blk = 0
                os = []
                for w in ws:
                    cap = caps[p][w]
                    ps = ppw.tile([128, C], F32, tag="psw")
                    for j in range(cap):
                        sel = sel_for(state["ci"])
                        state["ci"] += 1
                        nc.tensor.matmul(ps[:], lhsT=sel[:], rhs=stg[:, blk, :],
                                         start=(j == 0), stop=(j == cap - 1))
                        blk += 1
                    os.append((w, post_window(w, ps)))
                if tail is not None:
                    for w, o in os:
                        tail(w, o)

        def flush_partial(w, ps):
            """Phase-0 epilogue: partial = psum + self-term g[d]."""
            part = opool.tile([128, C], F32, tag=f"part_{w}")
            nc.vector.tensor_tensor(out=part[:], in0=ps[:], in1=gl[w][:],
                                    op=ALU.add)
            parts[w] = part

        def final_epilogue(w, ps, bias_sb):
            """Phase-1 epilogue: relu(dinv*(ps + partial) + bias)."""
            t1 = epool.tile([128, C], F32, tag="t1")
            nc.vector.tensor_tensor(out=t1[:], in0=ps[:], in1=parts[w][:],
                                    op=ALU.add)
            t2 = epool.tile([128, C], F32, tag="t2")
            nc.scalar.activation(t2[:], t1[:], AF.Identity,
                                 scale=dinvl[:, w:w + 1])
            t3 = epool.tile([128, C], F32, tag="t3")
            nc.vector.tensor_tensor(out=t3[:], in0=t2[:], in1=bias_sb[:],
                                    op=ALU.add)
            o = opool2.tile([128, C], F32, tag="otile")
            nc.scalar.activation(o[:], t3[:], AF.Relu)
            return o

        def l1_tail(w, o):
            """Transpose h1 and build the L2 table row for window w."""
            pst = ppt.tile([128, 128], F32, tag="pst")
            nc.tensor.transpose(pst[:], o[:], ident[:])
            h1t = epool.tile([128, 128], BF16, tag="h1t")
            nc.vector.tensor_copy(h1t[:], pst[:])
            ps2 = ppg.tile([128, C], F32, tag="psg")
            nc.tensor.matmul(ps2[:], lhsT=h1t[:], rhs=w2t[:],
                             start=True, stop=True)
            gb = epool.tile([128, C], BF16, tag="gb")
            nc.scalar.activation(gb[:], ps2[:], AF.Identity,
                                 scale=dinvl[:, w:w + 1])
            p = int(w >= W0)
            r0 = (w - (W0 if p else 0)) * 128
            nc.sync.dma_start(gloc[(2, p)][r0:r0 + 128, :], gb[:])
            glw = opool.tile([128, C], F32, tag=f"gl_{w}")
            nc.scalar.activation(glw[:], ps2[:], AF.Identity,
                                 scale=dinvl[:, w:w + 1])
            gl[w] = glw
            if w == W0 - 1:
                nc.gpsimd.collective_compute(
                    "AllGather", ALU.bypass,
                    replica_groups=[list(range(CORES))],
                    ins=[gloc[(2, 0)][:]], outs=[gtab[(2, 0)][:]])
            if w == NW - 1:
                nc.gpsimd.collective_compute(
                    "AllGather", ALU.bypass,
                    replica_groups=[list(range(CORES))],
                    ins=[gloc[(2, 1)][:]], outs=[gtab[(2, 1)][:]])

        def l2_tail(w, o):
            """Project and store output rows for window w."""
            pst = ppt.tile([128, 128], F32, tag="pst")
            nc.tensor.transpose(pst[:], o[:], ident[:])
            h2t = epool.tile([128, 128], BF16, tag="h2t")
            nc.vector.tensor_copy(h2t[:], pst[:])
            psp = ppp.tile([128, OUT_C], F32, tag="psp")
            nc.tensor.matmul(psp[:], lhsT=h2t[:], rhs=wpt[:],
                             start=True, stop=True)
            of = epool.tile([128, OUT_C], F32, tag="of")
            nc.vector.tensor_tensor(out=of[:], in0=psp[:], in1=bpb[:],
                                    op=ALU.add)
            ofr = epool.tile([128, OUT_C], F32, tag="ofr")
            nc.scalar.activation(ofr[:], of[:], AF.Relu)
            nc.sync.dma_start(out_d[w * 128:(w + 1) * 128, :], ofr[:])

        phase(1, 0, flush_partial)
        phase(1, 1, lambda w, ps: final_epilogue(w, ps, b1b), l1_tail)
        phase(2, 0, flush_partial)
        phase(2, 1, lambda w, ps: final_epilogue(w, ps, b2b), l2_tail)

    nc.compile()
    return nc


def _make_in_maps(sched, per_core, W1, b1, W2, b2, Wp, bp):
    w1t = np.ascontiguousarray(np.asarray(W1, np.float32).T).astype(NP_BF16)
    w2t = np.ascontiguousarray(np.asarray(W2, np.float32).T).astype(NP_BF16)
    wpt = np.ascontiguousarray(np.asarray(Wp, np.float32).T).astype(NP_BF16)
    b1b = np.ascontiguousarray(np.tile(np.asarray(b1, np.float32)[None], (128, 1)))
    b2b = np.ascontiguousarray(np.tile(np.asarray(b2, np.float32)[None], (128, 1)))
    bpb = np.ascontiguousarray(np.tile(np.asarray(bp, np.float32)[None], (128, 1)))
    iota = np.ascontiguousarray(
        np.tile(np.arange(128, dtype=np.float32)[None, :],
                (128, 1))).astype(NP_BF16)
    ident = np.eye(128, dtype=np.float32)
    base = dict(w1t=w1t, w2t=w2t, wpt=wpt, b1b=b1b, b2b=b2b, bpb=bpb,
                iota=iota, ident=ident)
    in_maps = []
    for c in range(CORES):
        pc = per_core[c]
        m = dict(base)
        m["idx"] = pc["idx"]
        m["rel"] = pc["rel"]
        m["degl"] = pc["degl"]
        m["xtl"] = pc["xtl"]
        in_maps.append(m)
    return in_maps


def _run(inputs, trace=False, tmpdir=None, verbose=True):
    import time
    t0 = time.time()
    def _log(msg):
        if verbose:
            print(f"[kernel {time.time()-t0:7.1f}s] {msg}", flush=True)
    sched, per_core = _host_prep(inputs["x"], inputs["edge_index"])
    _log("host prep done")
    nc = _build_nc(sched)
    _log("build+compile done")
    in_maps = _make_in_maps(sched, per_core,
                            inputs["W1"], inputs["b1"], inputs["W2"],
                            inputs["b2"], inputs["Wp"], inputs["bp"])
    _log("in_maps done")
    core_ids = list(range(CORES))
    if trace:
        # NTFF profiling needs a warm first execute; run once untraced.
        bass_utils.run_bass_kernel_spmd(nc, in_maps, core_ids=core_ids,
                                        trace=False)
        _log("warmup run done")
    res = bass_utils.run_bass_kernel_spmd(nc, in_maps, core_ids=core_ids,
                                          trace=trace, tmpdir=tmpdir)
    _log("run done")
    out = np.empty((N, OUT_C), np.float32)
    for c in range(CORES):
        out[c * S:(c + 1) * S] = res.results[c]["out"][:S]
    return out, res


def kernel(**inputs):
    out, _ = _run(inputs)
    return out


# revision 13
# speedup vs baseline: 1.5303x; 1.0249x over previous
"""GCN encoder (2x GCNConv + linear projection, relu) on 8 Trainium2 cores.

Self-contained: hardcodes the problem shapes (N=50000, E=800000, C=128,
OUT_C=64) and the sharding strategy.  Host side does structural prep only
(edge partitioning/sorting/padding, index-list construction); all FP math
(matmuls, rsqrt, scaling, aggregation, bias, relu) runs on device.

Math identity used on device, per GCNConv layer:
    g = dinv * (x @ W.T)          (dinv = rsqrt(indeg+1), per node)
    out[d] = relu(dinv[d] * (sum_{e: dst=d} g[src_e] + g[d]) + b)

Device mapping per core (v3):
  - nodes sharded by contiguous range (6250/core, padded to 6272 = 49
    windows of 128)
  - per layer the g table is built COLLECTIVELY: each core computes only
    its own 49 windows of g rows (local matmul + dinv scale); two
    AllGather collectives assemble two "piece" tables in DRAM:
      piece0 = all cores' windows 0..24  (8*3200 = 25600 rows)
      piece1 = all cores' windows 25..48 (8*3072 = 24576 rows)
    Both < 32768 rows so int16 gather indices cover them, and piece0 is
    ready early so its gathers overlap the piece1 collective.
  - edges partitioned by dst owner; per layer processed in two phases
    (piece0 edges, then piece1 edges).  Within a phase, windows are
    processed in groups of 4; ONE dma_gather per (group, piece) emitted
    as PREPARE_ONLY + trigger_dma so the GpSimd engine is never held
    through the transfer (descriptor generation runs even before the
    table collective lands; the trigger carries that dep), and calls
    rotate over the 4 SWDGE queues so their drains proceed concurrently.
  - segment-sum per window: selection-matrix (tensor_scalar is_equal vs
    iota) matmuls accumulating into a [128 dst x 128 feat] fp32 PSUM
    tile; phase-0 result (+ self term g[d]) is flushed to SBUF, phase-1
    re-accumulates and applies the epilogue.
  - PE tail work (transpose + next-layer table row / projection) is
    batched at group end so it does not stall the in-order PE queue
    behind each window's DVE epilogue.
"""

import sys
import numpy as np

for _p in ("/opt/trn_rl_repo",):
    if _p not in sys.path:
        sys.path.append(_p)

import concourse.bacc as bacc
import concourse.tile as tile
from concourse import bass, mybir, bass_utils

F32 = mybir.dt.float32
BF16 = mybir.dt.bfloat16
I16 = mybir.dt.int16
AF = mybir.ActivationFunctionType
ALU = mybir.AluOpType
NP_BF16 = mybir.dt.np(BF16)

N = 50000
E = 800000
C = 128
OUT_C = 64
CORES = 8
S = N // CORES            # 6250 real nodes per shard
NW = 49                   # windows of 128 dst nodes per core
SP = NW * 128             # 6272 padded shard rows
W0 = 25                   # windows in piece 0
W1 = NW - W0              # windows in piece 1
PR0 = W0 * 128            # 3200 piece-0 rows per core
PR1 = W1 * 128            # 3072
T0 = CORES * PR0          # 25600 piece-0 table rows
T1 = CORES * PR1          # 24576
WG = 4                    # windows per gather group
GROUPS = [list(range(s, min(s + WG, NW))) for s in range(0, NW, WG)]
NG = len(GROUPS)          # 13


def _wrap16(a):
    """[L] -> [128, L/16] int16 idx layout for dma_gather (16-wrap, 8x repl)."""
    assert a.size % 16 == 0
    w = a.reshape(-1, 16).T.astype(np.int16)
    return np.ascontiguousarray(np.tile(w, (8, 1)))


def _host_prep(x, edge_index):
    """Structural prep: edge partitioning/sorting/padding + index lists."""
    src = np.asarray(edge_index[0]).astype(np.int64)
    dst = np.asarray(edge_index[1]).astype(np.int64)
    deg = np.bincount(dst, minlength=N).astype(np.float32) + 1.0

    owner = dst // S
    loc = dst - owner * S
    win = loc // 128
    rel = (loc % 128).astype(np.float32)
    sc = src // S
    sl = src - sc * S
    pc = (sl >= PR0).astype(np.int64)            # source piece
    prow = np.where(pc == 1, sc * PR1 + sl - PR0, sc * PR0 + sl)

    # common chunk schedule: caps[p][w] = chunks per (window, piece),
    # max over cores so the single SPMD program fits every core
    key = ((owner * NW + win) * 2 + pc)
    counts = np.bincount(key, minlength=CORES * NW * 2).reshape(CORES, NW, 2)
    maxc = counts.max(axis=0)                                  # [NW, 2]
    caps = [[-(-int(maxc[w, p]) // 128) for w in range(NW)] for p in (0, 1)]
    assert all(cc >= 1 for p in (0, 1) for cc in caps[p])
    pad8 = [(-sum(caps[0])) % 8, (-sum(caps[1])) % 8]

    # per-(group, piece) gather call sizes (blocks) and idx column offsets
    gnb = [[sum(caps[p][w] for w in ws) for ws in GROUPS] for p in (0, 1)]
    off16 = {}
    o = 0
    for p in (0, 1):
        for g in range(NG):
            off16[(g, p)] = o
            o += gnb[p][g] * 8            # blocks*128/16 idx cols
    tot16 = o

    degp = np.ones(SP * CORES, np.float32)
    nodes = np.arange(N, dtype=np.int64)
    degp[(nodes // S) * SP + (nodes % S)] = deg

    per_core = []
    for c in range(CORES):
        m = owner == c
        cw, cr, cs, cp = win[m], rel[m], prow[m], pc[m]
        order = np.lexsort((cs, cp, cw))
        cw, cr, cs, cp = cw[order], cr[order], cs[order], cp[order]
        k = cw * 2 + cp
        idx_parts = [[], []]
        rel_parts = [[], []]
        for w in range(NW):
            for p in (0, 1):
                lo = np.searchsorted(k, w * 2 + p, "left")
                hi = np.searchsorted(k, w * 2 + p, "right")
                n = hi - lo
                cap = caps[p][w]
                assert n <= cap * 128
                iv = np.concatenate(
                    [cs[lo:hi], np.zeros(cap * 128 - n, np.int64)])
                rv = np.concatenate(
                    [cr[lo:hi], np.full(cap * 128 - n, -1.0, np.float32)])
                idx_parts[p].append(iv)
                rel_parts[p].append(rv)
        # idx stream in call order: (p, group)
        idx_all = []
        for p in (0, 1):
            for ws in GROUPS:
                for w in ws:
                    idx_all.append(idx_parts[p][w])
        idx_all = np.concatenate(idx_all)
        # rel stream in chunk-consumption order: phase 0 windows (+pad to
        # x8 columns for the batched is_equal), then phase 1
        rel_all = np.concatenate(
            rel_parts[0] + [np.full(pad8[0] * 128, -1.0, np.float32)] +
            rel_parts[1] + [np.full(pad8[1] * 128, -1.0, np.float32)])
        relT = np.ascontiguousarray(
            rel_all.reshape(-1, 128).T).astype(NP_BF16)
        degl = np.ascontiguousarray(
            degp[c * SP:(c + 1) * SP].reshape(NW, 128).T)        # [128, NW]
        xpad = np.zeros((SP, C), np.float32)
        xpad[:S] = np.asarray(x, np.float32)[c * S:(c + 1) * S]
        xtl = np.ascontiguousarray(xpad.T).astype(NP_BF16)       # [128, SP]
        per_core.append(dict(idx=_wrap16(idx_all), rel=relT, degl=degl,
                             xtl=xtl))

    sched = dict(caps=caps, gnb=gnb, off16=off16, tot16=tot16, pad8=pad8)
    return sched, per_core


def _build_nc(sched):
    caps, gnb, off16, tot16, pad8 = (sched["caps"], sched["gnb"],
                                     sched["off16"], sched["tot16"],
                                     sched["pad8"])
    nb_max = [max(gnb[0]), max(gnb[1])]
    nchcol = sum(caps[0]) + pad8[0] + sum(caps[1]) + pad8[1]

    nc = bacc.Bacc("TRN2", target_bir_lowering=False, debug=False,
                   enable_asserts=False, num_devices=CORES,
                   num_swdge_queues=4)

    def inp(name, shape, dt=F32):
        return nc.dram_tensor(name, shape, dt, kind="ExternalInput").ap()

    xtl_d = inp("xtl", [128, SP], BF16)
    w1t_d = inp("w1t", [C, C], BF16)
    w2t_d = inp("w2t", [C, C], BF16)
    wpt_d = inp("wpt", [C, OUT_C], BF16)
    b1b_d = inp("b1b", [128, C])
    b2b_d = inp("b2b", [128, C])
    bpb_d = inp("bpb", [128, OUT_C])
    degl_d = inp("degl", [128, NW])
    iota_d = inp("iota", [128, 8 * 128], BF16)
    ident_d = inp("ident", [128, 128])
    idx_d = inp("idx", [128, tot16], I16)
    rel_d = inp("rel", [128, nchcol], BF16)
    out_d = nc.dram_tensor("out", [SP, OUT_C], F32, kind="ExternalOutput").ap()

    gloc = {}
    gtab = {}
    for L in (1, 2):
        for p, (pr, tr) in enumerate(((PR0, T0), (PR1, T1))):
            gloc[(L, p)] = nc.dram_tensor(
                f"g{L}loc{p}", [pr, C], BF16, kind="Internal").ap()
            gtab[(L, p)] = nc.dram_tensor(
                f"g{L}tab{p}", [tr, C], BF16, kind="Internal",
                addr_space="Shared").ap()

    from contextlib import ExitStack
    with tile.TileContext(nc) as tc, ExitStack() as ctx:
        cp = ctx.enter_context(tc.tile_pool(name="consts", bufs=1))
        stg0 = ctx.enter_context(tc.tile_pool(name="stg0", bufs=4))
        stg1 = ctx.enter_context(tc.tile_pool(name="stg1", bufs=4))
        spool = ctx.enter_context(tc.tile_pool(name="sel", bufs=6))
        epool = ctx.enter_context(tc.tile_pool(name="epi", bufs=4))
        opool = ctx.enter_context(tc.tile_pool(name="otiles", bufs=1))
        opool2 = ctx.enter_context(tc.tile_pool(name="owin", bufs=6))
        ppw = ctx.enter_context(tc.tile_pool(name="psw", bufs=2, space="PSUM"))
        ppg = ctx.enter_context(tc.tile_pool(name="psg", bufs=2, space="PSUM"))
        ppt = ctx.enter_context(tc.tile_pool(name="pst", bufs=2, space="PSUM"))
        ppp = ctx.enter_context(tc.tile_pool(name="psp", bufs=1, space="PSUM"))

        def cload(name, ap, shape, dt=F32):
            t = cp.tile(shape, dt, tag=name)
            nc.sync.dma_start(t[:], ap[:])
            return t

        # build-critical consts first so the first collective fires ASAP
        degl = cload("degl", degl_d, [128, NW])
        xtl = cload("xtl", xtl_d, [128, SP], BF16)
        w1t = cload("w1t", w1t_d, [C, C], BF16)
        w2t = cload("w2t", w2t_d, [C, C], BF16)
        sql = cp.tile([128, NW], F32, tag="sql")
        nc.scalar.activation(sql[:], degl[:], AF.Sqrt)
        dinvl = cp.tile([128, NW], F32, tag="dinvl")
        nc.vector.reciprocal(dinvl[:], sql[:])

        # ---- L1 local table rows: g1 = dinv * (X_local @ W1.T) ----
        gl = {}
        for w in range(NW):
            ps = ppg.tile([128, C], F32, tag="psg")
            nc.tensor.matmul(ps[:], lhsT=xtl[:, w * 128:(w + 1) * 128],
                             rhs=w1t[:], start=True, stop=True)
            gb = epool.tile([128, C], BF16, tag="gb")
            nc.scalar.activation(gb[:], ps[:], AF.Identity,
                                 scale=dinvl[:, w:w + 1])
            p = int(w >= W0)
            r0 = (w - (W0 if p else 0)) * 128
            nc.sync.dma_start(gloc[(1, p)][r0:r0 + 128, :], gb[:])
            glw = opool.tile([128, C], F32, tag=f"gl_{w}")
            nc.scalar.activation(glw[:], ps[:], AF.Identity,
                                 scale=dinvl[:, w:w + 1])
            gl[w] = glw
            if w == W0 - 1:
                nc.gpsimd.collective_compute(
                    "AllGather", ALU.bypass,
                    replica_groups=[list(range(CORES))],
                    ins=[gloc[(1, 0)][:]], outs=[gtab[(1, 0)][:]])
        nc.gpsimd.collective_compute(
            "AllGather", ALU.bypass, replica_groups=[list(range(CORES))],
            ins=[gloc[(1, 1)][:]], outs=[gtab[(1, 1)][:]])

        # remaining consts (overlap the first collective)
        wpt = cload("wpt", wpt_d, [C, OUT_C], BF16)
        b1b = cload("b1b", b1b_d, [128, C])
        b2b = cload("b2b", b2b_d, [128, C])
        bpb = cload("bpb", bpb_d, [128, OUT_C])
        iota = cload("iota", iota_d, [128, 8 * 128], BF16)
        ident = cload("ident", ident_d, [128, 128])
        idx = cload("idx", idx_d, [128, tot16], I16)
        rel = cload("rel", rel_d, [128, nchcol], BF16)

        state = dict(ci=0, qi=0)
        parts = {}

        selb_cur = [None]

        def sel_for(ci):
            if ci % 8 == 0:
                sb = spool.tile([128, 8 * 128], BF16, tag="selb")
                nc.vector.tensor_tensor(
                    out=sb[:].rearrange("p (c f) -> p c f", f=128),
                    in0=rel[:, ci:ci + 8].rearrange(
                        "p (c o) -> p c o", o=1).to_broadcast([128, 8, 128]),
                    in1=iota[:].rearrange("p (c f) -> p c f", f=128),
                    op=ALU.is_equal)
                selb_cur[0] = sb
            k = ci % 8
            return selb_cur[0][:, k * 128:(k + 1) * 128]

        def phase(L, p, post_window, tail=None):
            """One gather+aggregate phase: piece p edges of layer L."""
            if p == 0:
                state["ci"] = 0          # rel stream is shared by both layers
            pool = stg0 if p == 0 else stg1
            for g, ws in enumerate(GROUPS):
                nb = gnb[p][g]
                stg = pool.tile([128, nb_max[p], C], BF16, tag="stg")
                nidx = nb * 128
                o16 = off16[(g, p)]
                q = state["qi"] % 4
                state["qi"] += 1
                nc.gpsimd.dma_gather(
                    stg[:, :nb, :], gtab[(L, p)],
                    idx[:, o16:o16 + nidx // 16], nidx, nidx, elem_size=C,
                    single_packet=False, queue_num=q)
                blk = 0
                os = []
                for w in ws:
                    cap = caps[p][w]
                    ps = ppw.tile([128, C], F32, tag="psw")
                    for j in range(cap):
                        sel = sel_for(state["ci"])
                        state["ci"] += 1
                        nc.tensor.matmul(ps[:], lhsT=sel, rhs=stg[:, blk, :],
                                         start=(j == 0), stop=(j == cap - 1))
                        blk += 1
                    os.append((w, post_window(w, ps)))
                if tail is not None:
                    for w, o in os:
                        tail(w, o)
            state["ci"] += pad8[p]

        def flush_partial(w, ps):
            """Phase-0 epilogue: partial = psum + self-term g[d]."""
            part = opool.tile([128, C], F32, tag=f"part_{w}")
            nc.vector.tensor_tensor(out=part[:], in0=ps[:], in1=gl[w][:],
                                    op=ALU.add)
            parts[w] = part

        def final_epilogue(w, ps, bias_sb):
            """Phase-1 epilogue: relu(dinv*(ps + partial) + bias)."""
            t1 = epool.tile([128, C], F32, tag="t1")
            nc.vector.tensor_tensor(out=t1[:], in0=ps[:], in1=parts[w][:],
                                    op=ALU.add)
            t2 = epool.tile([128, C], F32, tag="t2")
            nc.scalar.activation(t2[:], t1[:], AF.Identity,
                                 scale=dinvl[:, w:w + 1])
            t3 = epool.tile([128, C], F32, tag="t3")
            nc.vector.tensor_tensor(out=t3[:], in0=t2[:], in1=bias_sb[:],
                                    op=ALU.add)
            o = opool2.tile([128, C], F32, tag="otile")
            nc.scalar.activation(o[:], t3[:], AF.Relu)
            return o

        def l1_tail(w, o):
            """Transpose h1 and build the L2 table row for window w."""
            pst = ppt.tile([128, 128], F32, tag="pst")
            nc.tensor.transpose(pst[:], o[:], ident[:])
            h1t = epool.tile([128, 128], BF16, tag="h1t")
            nc.vector.tensor_copy(h1t[:], pst[:])
            ps2 = ppg.tile([128, C], F32, tag="psg")
            nc.tensor.matmul(ps2[:], lhsT=h1t[:], rhs=w2t[:],
                             start=True, stop=True)
            gb = epool.tile([128, C], BF16, tag="gb")
            nc.scalar.activation(gb[:], ps2[:], AF.Identity,
                                 scale=dinvl[:, w:w + 1])
            p = int(w >= W0)
            r0 = (w - (W0 if p else 0)) * 128
            nc.sync.dma_start(gloc[(2, p)][r0:r0 + 128, :], gb[:])
            glw = opool.tile([128, C], F32, tag=f"gl_{w}")
            nc.scalar.activation(glw[:], ps2[:], AF.Identity,
                                 scale=dinvl[:, w:w + 1])
            gl[w] = glw
            if w == W0 - 1:
                nc.gpsimd.collective_compute(
                    "AllGather", ALU.bypass,
                    replica_groups=[list(range(CORES))],
                    ins=[gloc[(2, 0)][:]], outs=[gtab[(2, 0)][:]])
            if w == NW - 1:
                nc.gpsimd.collective_compute(
                    "AllGather", ALU.bypass,
                    replica_groups=[list(range(CORES))],
                    ins=[gloc[(2, 1)][:]], outs=[gtab[(2, 1)][:]])

        def l2_tail(w, o):
            """Project and store output rows for window w."""
            pst = ppt.tile([128, 128], F32, tag="pst")
            nc.tensor.transpose(pst[:], o[:], ident[:])
            h2t = epool.tile([128, 128], BF16, tag="h2t")
            nc.vector.tensor_copy(h2t[:], pst[:])
            psp = ppp.tile([128, OUT_C], F32, tag="psp")
            nc.tensor.matmul(psp[:], lhsT=h2t[:], rhs=wpt[:],
                             start=True, stop=True)
            of = epool.tile([128, OUT_C], F32, tag="of")
            nc.vector.tensor_tensor(out=of[:], in0=psp[:], in1=bpb[:],
                                    op=ALU.add)
            ofr = epool.tile([128, OUT_C], F32, tag="ofr")
            nc.scalar.activation(ofr[:], of[:], AF.Relu)
            nc.sync.dma_start(out_d[w * 128:(w + 1) * 128, :], ofr[:])

        phase(1, 0, flush_partial)
        phase(1, 1, lambda w, ps: final_epilogue(w, ps, b1b), l1_tail)
        phase(2, 0, flush_partial)
        phase(2, 1, lambda w, ps: final_epilogue(w, ps, b2b), l2_tail)

    nc.compile()
    return nc


def _make_in_maps(sched, per_core, W1, b1, W2, b2, Wp, bp):
    w1t = np.ascontiguousarray(np.asarray(W1, np.float32).T).astype(NP_BF16)
    w2t = np.ascontiguousarray(np.asarray(W2, np.float32).T).astype(NP_BF16)
    wpt = np.ascontiguousarray(np.asarray(Wp, np.float32).T).astype(NP_BF16)
    b1b = np.ascontiguousarray(np.tile(np.asarray(b1, np.float32)[None], (128, 1)))
    b2b = np.ascontiguousarray(np.tile(np.asarray(b2, np.float32)[None], (128, 1)))
    bpb = np.ascontiguousarray(np.tile(np.asarray(bp, np.float32)[None], (128, 1)))
    iota = np.ascontiguousarray(
        np.tile(np.arange(128, dtype=np.float32)[None, :],
                (128, 8))).astype(NP_BF16)
    ident = np.eye(128, dtype=np.float32)
    base = dict(w1t=w1t, w2t=w2t, wpt=wpt, b1b=b1b, b2b=b2b, bpb=bpb,
                iota=iota, ident=ident)
    in_maps = []
    for c in range(CORES):
        pc = per_core[c]
        m = dict(base)
        m["idx"] = pc["idx"]
        m["rel"] = pc["rel"]
        m["degl"] = pc["degl"]
        m["xtl"] = pc["xtl"]
        in_maps.append(m)
    return in_maps


def _run(inputs, trace=False, tmpdir=None, verbose=True):
    import time
    t0 = time.time()
    def _log(msg):
        if verbose:
            print(f"[kernel {time.time()-t0:7.1f}s] {msg}", flush=True)
    sched, per_core = _host_prep(inputs["x"], inputs["edge_index"])
    _log("host prep done")
    nc = _build_nc(sched)
    _log("build+compile done")
    in_maps = _make_in_maps(sched, per_core,
                            inputs["W1"], inputs["b1"], inputs["W2"],
                            inputs["b2"], inputs["Wp"], inputs["bp"])
    _log("in_maps done")
    core_ids = list(range(CORES))
    if trace:
        # NTFF profiling needs a warm first execute; run once untraced.
        bass_utils.run_bass_kernel_spmd(nc, in_maps, core_ids=core_ids,
                                        trace=False)
        _log("warmup run done")
    res = bass_utils.run_bass_kernel_spmd(nc, in_maps, core_ids=core_ids,
                                          trace=trace, tmpdir=tmpdir)
    _log("run done")
    out = np.empty((N, OUT_C), np.float32)
    for c in range(CORES):
        out[c * S:(c + 1) * S] = res.results[c]["out"][:S]
    return out, res


def kernel(**inputs):
    out, _ = _run(inputs)
    return out
